# revision 48
# baseline (speedup 1.0000x reference)
"""Distributed Trainium2 Bass kernel for nn_CausalSelfAttention_66984309948568.

Strategy (8 NeuronCores, tensor-parallel over heads):
  - core h owns head h (8 heads, head_dim 128). All matmul operands bf16
    (full PE rate, half the DMA/SBUF traffic of f32); PSUM accumulation f32.
  - Phase 0 (per 512-wide t-chunk): qkv projection in [d, t] layout from a
    bf16 x^T chunk; v transposed to [t, d] tiles and fused with lambdas[1]*ve;
    q, k roped UNNORMALIZED (rope commutes with the per-t rms_norm scale).
    Row sums of q^2 / k^2 via ones-column matmuls, staged into [1, T] rows.
    Per-chunk Ln of the mean-square rows; a manually preloaded act-table
    set holding Ln+Exp+Copy makes every activation reload-free:
    rsq = exp(-0.5*ln(mean+eps) [+ ln 0.12 for q]).
  - Phase 2 (per chunk): normalize q/k chunk via ones-row broadcast matmul +
    elementwise mult; causal attention streamed per 512-column chunk in
    S^T = [s, t] layout (no max subtraction: |scores| <= 15.4); softmax
    denominator via accumulated ones-column matmul over exp(S^T); y
    normalized with broadcast reciprocal; bf16 result staged for AllToAll.
  - AllToAll (bf16) exchanges per-head y slices; each core projects its own
    512 output rows; host concatenates.
"""

import sys

sys.path.insert(0, "/opt/trn_rl_repo")

import numpy as np
import concourse.bass as bass
import concourse.bacc as bacc
import concourse.mybir as mybir
from concourse import tile
from concourse.bass_utils import run_bass_kernel_spmd

N_CORES = 8
B, T, DIM = 1, 4096, 1024
NUM_HEADS, HEAD_DIM = 8, 128
HDIM = NUM_HEADS * HEAD_DIM
SCALE = 0.12
EPS = 1.1920928955078125e-07
NCHUNK = T // 512          # 8 t-chunks of 512
NTT = T // 128             # 32 t-tiles of 128
TSLICE = T // N_CORES      # 512 output rows per core

f32 = mybir.dt.float32
bf16 = mybir.dt.bfloat16
FN = mybir.ActivationFunctionType
ALU = mybir.AluOpType
MASK_NEG = -30000.0


def _register_const(nc, value, dtype=f32):
    if (dtype, value) in nc.const_aps.aps:
        return
    t = nc.alloc_sbuf_tensor(f"const-{dtype.name}-{value}", [128, 1], dtype)
    nc.gpsimd.memset(t.ap(), value)
    nc.const_aps.aps[(dtype, value)] = t.ap()


def _act_combined_set_id(nc):
    # Index of the act-table set containing BOTH Ln and Exp (plus Copy):
    # preloading it lets the act-table pass skip every per-function reload
    # regardless of how the scheduler interleaves Ln/Exp.
    try:
        from concourse.hw_specs import get_activation_tables

        tabs = get_activation_tables(nc.m.arch)
        for idx, (name, funcs) in enumerate(tabs.items()):
            if FN.Ln in funcs and FN.Exp in funcs and FN.Copy in funcs:
                return idx
    except Exception:
        pass
    return 6  # natural_log_exp_and_others in the gen3 act_info.json


def _build_program(repeat=1):
    nc = bacc.Bacc(num_devices=N_CORES)
    _register_const(nc, EPS)
    _register_const(nc, float(np.log(SCALE)))
    _register_const(nc, 0.0)
    nc.all_engine_barrier()
    nc.scalar.add_instruction(
        mybir.InstLoadActFuncSet(
            name=nc.get_next_instruction_name(),
            act_func_set_id=_act_combined_set_id(nc),
            ins=[],
            outs=[],
        )
    )

    ln_scale_q = float(np.log(SCALE))

    # ---- DRAM parameters (per-core values supplied via in_maps) ----
    xt_d = nc.declare_dram_parameter("xt", [DIM, T], bf16, isOutput=False)
    wq_d = nc.declare_dram_parameter("wq", [128, DIM], bf16, isOutput=False)
    wk_d = nc.declare_dram_parameter("wk", [128, DIM], bf16, isOutput=False)
    wv_d = nc.declare_dram_parameter("wv", [128, DIM], bf16, isOutput=False)
    vew_d = nc.declare_dram_parameter("vew", [128, T], bf16, isOutput=False)
    cmat_d = nc.declare_dram_parameter("cmat", [128, T], bf16, isOutput=False)
    smat_d = nc.declare_dram_parameter("smat", [128, T], bf16, isOutput=False)
    mask_d = nc.declare_dram_parameter("maskc", [128, 128], f32, isOutput=False)
    pw_d = nc.declare_dram_parameter("pw", [128, 8 * DIM], bf16, isOutput=False)
    onc_d = nc.declare_dram_parameter("ones_col", [128, 1], bf16, isOutput=False)
    onr_d = nc.declare_dram_parameter("ones_row", [1, 128], bf16, isOutput=False)
    id_d = nc.declare_dram_parameter("ident", [128, 128], bf16, isOutput=False)
    out_d = nc.declare_dram_parameter("out", [TSLICE, DIM], f32, isOutput=True)

    with tile.TileContext(nc, num_cores=N_CORES) as tc:
        with (
            tc.tile_pool(name="persist", bufs=1) as persist,
            tc.tile_pool(name="dram", bufs=1, space="DRAM") as dram,
        ):
            # persistent SBUF tensors
            qT = persist.tile([128, T], bf16, tag="qT")       # rope(q), unnormalized
            kT = persist.tile([128, T], bf16, tag="kT")
            qnT = persist.tile([128, T], bf16, tag="qnT")     # 0.12 * norm-scaled
            knT = persist.tile([128, T], bf16, tag="knT")
            v_sb = persist.tile([128, NTT * 128], bf16, tag="v_sb")  # [t,d] tiles
            maskc = persist.tile([128, 128], f32, tag="maskc")
            onc = persist.tile([128, 1], bf16, tag="onc")
            onr = persist.tile([1, 128], bf16, tag="onr")
            ident = persist.tile([128, 128], bf16, tag="ident")
            lnr_q = persist.tile([1, T], f32, tag="lnr_q")    # ln(mean sq) rows
            lnr_k = persist.tile([1, T], f32, tag="lnr_k")
            rsq_q = persist.tile([1, T], bf16, tag="rsq_q")
            rsq_k = persist.tile([1, T], bf16, tag="rsq_k")
            pw = persist.tile([128, 8 * DIM], bf16, tag="pw")

            nc.gpsimd.dma_start(maskc[:], mask_d[:])
            nc.gpsimd.dma_start(onc[:], onc_d[:])
            nc.gpsimd.dma_start(onr[:], onr_d[:])
            nc.gpsimd.dma_start(ident[:], id_d[:])
            nc.sync.dma_start(pw[:], pw_d[:])

            a2a_in = dram.tile([N_CORES * 128, TSLICE], bf16, tag="a2a_in")
            a2a_out = dram.tile([N_CORES * 128, TSLICE], bf16, tag="a2a_out")

            # constant operands: loaded once, reused by every repeat
            wq = persist.tile([128, DIM], bf16, tag="wq")
            wk = persist.tile([128, DIM], bf16, tag="wk")
            wv = persist.tile([128, DIM], bf16, tag="wv")
            cmat = persist.tile([128, T], bf16, tag="cmat")
            smat = persist.tile([128, T], bf16, tag="smat")
            vew = persist.tile([128, T], bf16, tag="vew")
            ut = persist.tile([128, 512], bf16, tag="ut")
            nc.sync.dma_start(wq[:], wq_d[:])
            nc.sync.dma_start(wk[:], wk_d[:])
            nc.sync.dma_start(wv[:], wv_d[:])
            nc.gpsimd.dma_start(cmat[:], cmat_d[:])
            nc.gpsimd.dma_start(smat[:], smat_d[:])
            nc.gpsimd.dma_start(vew[:], vew_d[:])
            # rows 32:64 and 96:128 of ut stay zero forever: the rope
            # cross-term add runs over all 128 partitions in one op.
            nc.vector.memset(ut[:], 0.0)

            for _rep in range(repeat):
              # ============ Phase 0: qkv, rope (unnormalized), v ============
              with (
                  tc.tile_pool(name="xt", bufs=2) as xt_pool,
                  tc.tile_pool(name="qkv_ps", bufs=1, space=bass.MemorySpace.PSUM) as qkv_ps,
                  tc.tile_pool(name="row_ps", bufs=2, space=bass.MemorySpace.PSUM) as row_ps,
                  tc.tile_pool(name="tr_ps", bufs=2, space=bass.MemorySpace.PSUM) as tr_ps,
                  tc.tile_pool(name="evac", bufs=3) as evac,
                  tc.tile_pool(name="tmps", bufs=2) as tmps,
              ):

                  for c in range(NCHUNK):
                      cs = bass.ts(c, 512)
                      ps_q = qkv_ps.tile([128, 512], f32, tag="ps_q")
                      ps_k = qkv_ps.tile([128, 512], f32, tag="ps_k")
                      ps_v = qkv_ps.tile([128, 512], f32, tag="ps_v")
                      # one 1MB DMA per chunk: [p, dt, col] <- xt[128*dt + p, 512c + col]
                      xt_t = xt_pool.tile([128, 8, 512], bf16, tag="xt")
                      nc.sync.dma_start(
                          xt_t[:],
                          xt_d[:, 512 * c : 512 * (c + 1)].rearrange(
                              "(dt p) col -> p dt col", p=128
                          ),
                      )
                      for dt in range(8):
                          st, sp = dt == 0, dt == 7
                          nc.tensor.matmul(ps_q[:], wq[:, bass.ts(dt, 128)], xt_t[:, dt, :], start=st, stop=sp)
                          nc.tensor.matmul(ps_k[:], wk[:, bass.ts(dt, 128)], xt_t[:, dt, :], start=st, stop=sp)
                          nc.tensor.matmul(ps_v[:], wv[:, bass.ts(dt, 128)], xt_t[:, dt, :], start=st, stop=sp)

                      # ---- v: transpose [d,t]->[t,d] per 128-tile, add ve ----
                      vTc = evac.tile([128, 512], bf16, tag="vTc")
                      nc.scalar.copy(vTc[:], ps_v[:])
                      for j in range(4):
                          i = 4 * c + j
                          ps_t = tr_ps.tile([128, 128], bf16, tag="ps_t")
                          nc.tensor.transpose(ps_t[:], vTc[:, bass.ts(j, 128)], ident[:])
                          nc.vector.tensor_tensor(
                              v_sb[:, bass.ts(i, 128)], ps_t[:], vew[:, bass.ts(i, 128)], ALU.add
                          )

                      # ---- q, k: evac, square-rowsum, rope (unnormalized) ----
                      for which, ps_x, dstT in (("q", ps_q, qT), ("k", ps_k, kT)):
                          xTc = evac.tile([128, 512], bf16, tag="xTc")
                          nc.scalar.copy(xTc[:], ps_x[:])
                          sqc = tmps.tile([128, 512], bf16, tag="sqc")
                          nc.vector.tensor_tensor(sqc[:], xTc[:], xTc[:], ALU.mult)
                          ps_row = row_ps.tile([1, 512], f32, tag="ps_row")
                          nc.tensor.matmul(ps_row[:], onc[:], sqc[:], start=True, stop=True)
                          # ln(mean+eps) straight from PSUM; Ln lives in the
                          # same act table set as Copy, so phase 0 needs one
                          # table load total.
                          lnr_x = lnr_q if which == "q" else lnr_k
                          nc.scalar.activation(
                              lnr_x[:, cs], ps_row[:], FN.Ln,
                              bias=EPS, scale=1.0 / HEAD_DIM,
                          )
                          # rope: dstT = x*cmat + shift64(x)*smat
                          nc.vector.tensor_tensor(dstT[:, cs], xTc[:], cmat[:, cs], ALU.mult)
                          nc.vector.tensor_tensor(ut[0:32, :], xTc[64:96, :], smat[64:96, cs], ALU.mult)
                          nc.vector.tensor_tensor(ut[64:96, :], xTc[0:32, :], smat[0:32, cs], ALU.mult)
                          nc.vector.tensor_tensor(dstT[:, cs], dstT[:, cs], ut[:], ALU.add)

              # ================= Phase 2: causal attention ====================
              with (
                  tc.tile_pool(name="s_ps", bufs=4, space=bass.MemorySpace.PSUM) as s_ps,
                  tc.tile_pool(name="y_ps", bufs=2, space=bass.MemorySpace.PSUM) as y_ps,
                  tc.tile_pool(name="r_ps", bufs=1, space=bass.MemorySpace.PSUM) as r_ps,
                  tc.tile_pool(name="b_ps", bufs=1, space=bass.MemorySpace.PSUM) as b_ps,
                  tc.tile_pool(name="pt", bufs=4) as pt_pool,
                  tc.tile_pool(name="att_sb", bufs=2) as att_sb,
              ):
                  def normalize_chunk(cn):
                      """rsq = exp(-0.5*lnr [+ln 0.12]); qnT/knT = qT/kT * rsq."""
                      ns = bass.ts(cn, 512)
                      for lnr_x, bias, rsq, src, dst in (
                          (lnr_q, ln_scale_q, rsq_q, qT, qnT),
                          (lnr_k, 0.0, rsq_k, kT, knT),
                      ):
                          nc.scalar.activation(
                              rsq[:, ns], lnr_x[:, ns], FN.Exp,
                              bias=bias, scale=-0.5,
                          )
                          ps_b = b_ps.tile([128, 512], f32, tag="ps_b")
                          nc.tensor.matmul(ps_b[:], onr[:], rsq[:, ns], start=True, stop=True)
                          bb = att_sb.tile([128, 512], bf16, tag="bb")
                          nc.scalar.copy(bb[:], ps_b[:])
                          nc.vector.tensor_tensor(dst[:, ns], src[:, ns], bb[:], ALU.mult)

                  normalize_chunk(0)
                  normalize_chunk(1)
                  for c in range(NCHUNK):
                      cs = bass.ts(c, 512)
                      # normalize two chunks ahead: overlaps this chunk's
                      # attention and keeps the next chunks' qnT/knT ready
                      # well before their S matmuls.
                      if c + 2 < NCHUNK:
                          normalize_chunk(c + 2)

                      n_s = 4 * (c + 1)
                      ps_y = y_ps.tile([128, 512], f32, tag="ps_y")
                      ps_r = r_ps.tile([1, 512], f32, tag="ps_r")
                      # s-tile-granular software pipeline: S/exp of tile i
                      # run 2 tiles ahead of the y/r accumulation matmuls so
                      # the PE never stalls on the ACT exp latency.
                      DEPTH = 3
                      pts = {}

                      def emit_S(i):
                          # for a diagonal s-tile (j = i-4c >= 0) only the
                          # columns t_local >= 128j can be unmasked: stream the
                          # partial window and add the 128x128 triangle mask.
                          k_idx = i - 4 * c
                          o = 128 * k_idx if k_idx > 0 else 0
                          ps_S = s_ps.tile([128, 512], f32, tag="ps_S")
                          nc.tensor.matmul(
                              ps_S[:, o:512],
                              knT[:, bass.ts(i, 128)],
                              qnT[:, 512 * c + o : 512 * (c + 1)],
                              start=True, stop=True,
                          )
                          if k_idx >= 0:
                              nc.vector.tensor_tensor(
                                  ps_S[:, o : o + 128],
                                  ps_S[:, o : o + 128],
                                  maskc[:, 0:128],
                                  ALU.add,
                              )
                          pT = pt_pool.tile([128, 512], bf16, tag="pT")
                          nc.scalar.activation(pT[:, o:512], ps_S[:, o:512], FN.Exp)
                          pts[i] = (pT, o)

                      for i in range(min(DEPTH, n_s)):
                          emit_S(i)
                      for i in range(n_s):
                          if i + DEPTH < n_s:
                              emit_S(i + DEPTH)
                          pT, o = pts.pop(i)
                          st, sp = i == 0, i == n_s - 1
                          nc.tensor.matmul(
                              ps_y[:, o:512], v_sb[:, bass.ts(i, 128)], pT[:, o:512],
                              start=st, stop=sp,
                          )
                          nc.tensor.matmul(
                              ps_r[:, o:512], onc[:], pT[:, o:512], start=st, stop=sp
                          )
                      # normalize y chunk by 1/rowsum and ship to a2a buffer
                      rrec = att_sb.tile([1, 512], f32, tag="rrec")
                      nc.vector.reciprocal(rrec[:], ps_r[:])
                      rrecr = att_sb.tile([1, 512], bf16, tag="rrecr")
                      nc.vector.tensor_copy(rrecr[:], rrec[:])
                      ps_br = b_ps.tile([128, 512], f32, tag="ps_b")
                      nc.tensor.matmul(ps_br[:], onr[:], rrecr[:], start=True, stop=True)
                      brs = att_sb.tile([128, 512], f32, tag="brs")
                      nc.scalar.copy(brs[:], ps_br[:])
                      yn = att_sb.tile([128, 512], bf16, tag="yn")
                      nc.vector.tensor_tensor(yn[:], ps_y[:], brs[:], ALU.mult)
                      nc.scalar.dma_start(a2a_in[128 * c : 128 * (c + 1), :], yn[:])

              nc.gpsimd.collective_compute(
                  "AllToAll",
                  ALU.bypass,
                  replica_groups=[list(range(N_CORES))],
                  ins=[a2a_in[:].opt()],
                  outs=[a2a_out[:].opt()],
              )

              # ================= Phase 3: output projection =====================
              with (
                  tc.tile_pool(name="proj_sb", bufs=1) as proj_sb,
                  tc.tile_pool(name="o_ps", bufs=2, space=bass.MemorySpace.PSUM) as o_ps,
                  tc.tile_pool(name="outp", bufs=3) as outp,
              ):
                  yT = proj_sb.tile([128, N_CORES * TSLICE], bf16, tag="yT")
                  nc.sync.dma_start(
                      yT[:].rearrange("p (h t) -> p h t", h=N_CORES),
                      a2a_out[:].rearrange("(h p) t -> p h t", p=128),
                  )
                  for m in range(4):
                      for dc in range(2):
                          ps_o = o_ps.tile([128, 512], f32, tag="ps_o")
                          for hh in range(8):
                              nc.tensor.matmul(
                                  ps_o[:],
                                  yT[:, hh * TSLICE + 128 * m : hh * TSLICE + 128 * (m + 1)],
                                  pw[:, hh * DIM + 512 * dc : hh * DIM + 512 * (dc + 1)],
                                  start=(hh == 0), stop=(hh == 7),
                              )
                          ob = outp.tile([128, 512], f32, tag="ob")
                          nc.vector.tensor_copy(ob[:], ps_o[:])
                          nc.scalar.dma_start(
                              out_d[128 * m : 128 * (m + 1), 512 * dc : 512 * (dc + 1)], ob[:]
                          )

    nc.finalize()
    return nc


_PROGRAM = None


def _get_program():
    global _PROGRAM
    if _PROGRAM is None:
        _PROGRAM = _build_program()
    return _PROGRAM


def _bf16(a):
    import ml_dtypes

    return np.asarray(a, dtype=np.float32).astype(ml_dtypes.bfloat16)


def _host_prep(x, ve, qkv_w, lambdas, proj_w):
    x = np.asarray(x, dtype=np.float32).reshape(T, DIM)
    ve = np.asarray(ve, dtype=np.float32).reshape(T, HDIM)
    qkv_w = np.asarray(qkv_w, dtype=np.float32)
    lam = np.asarray(lambdas, dtype=np.float32)
    proj_w = np.asarray(proj_w, dtype=np.float32)

    xt = _bf16(x.T)                                        # [DIM, T]

    # rope tables
    nfreq = HEAD_DIM // 4
    ang = (1.0 / 1024.0) ** np.linspace(0.0, 1.0, nfreq, dtype=np.float32)
    theta = np.arange(T, dtype=np.float32)[:, None] * ang[None, :]     # [T, 32]
    cosT = np.cos(theta).T.astype(np.float32)              # [32, T]
    sinT = np.sin(theta).T.astype(np.float32)
    cmat = np.empty((128, T), np.float32)
    smat = np.empty((128, T), np.float32)
    cmat[0:32] = cosT
    cmat[32:64] = 1.0
    cmat[64:96] = cosT
    cmat[96:128] = 1.0
    smat[0:32] = -sinT
    smat[32:64] = 0.0
    smat[64:96] = sinT
    smat[96:128] = 0.0

    # triangle mask for the diagonal 128x128 blocks (partial-window streams
    # never touch the fully-masked region to the left of the triangle)
    maskc = np.where(
        np.arange(128)[:, None] > np.arange(128)[None, :], MASK_NEG, 0.0
    ).astype(np.float32)

    ones_col = _bf16(np.ones((128, 1), np.float32))
    ones_row = _bf16(np.ones((1, 128), np.float32))
    ident = _bf16(np.eye(128, dtype=np.float32))

    in_maps = []
    for h in range(N_CORES):
        hs = slice(128 * h, 128 * (h + 1))
        # weight layout: w[p, dt*128 + m] = W[m, dt*128 + p]
        Wq = qkv_w[0, hs, :]                                # [128, DIM]
        Wk = qkv_w[1, hs, :]
        Wv = qkv_w[2, hs, :] * lam[0]
        def wlay(W):
            # [m, (dt p)] -> [p, (dt m)]
            a = W.reshape(128, 8, 128)                      # [m, dt, p]
            return _bf16(np.ascontiguousarray(a.transpose(2, 1, 0).reshape(128, DIM)))
        # vew[p, i*128 + c] = lam1 * ve[i*128 + p, h*128 + c]
        veh = (ve[:, hs] * lam[1]).reshape(NTT, 128, 128)   # [i, p, c]
        vew = np.ascontiguousarray(veh.transpose(1, 0, 2).reshape(128, T))
        # pw[p, n*DIM + D] = proj_w[D, 128n + p]
        pwh = proj_w.T.reshape(8, 128, DIM)                 # [n, e_p, D]
        pw = _bf16(np.ascontiguousarray(pwh.transpose(1, 0, 2).reshape(128, 8 * DIM)))
        in_maps.append(
            {
                "xt": xt,
                "wq": wlay(Wq),
                "wk": wlay(Wk),
                "wv": wlay(Wv),
                "vew": _bf16(vew),
                "cmat": _bf16(cmat),
                "smat": _bf16(smat),
                "maskc": maskc,
                "pw": pw,
                "ones_col": ones_col,
                "ones_row": ones_row,
                "ident": ident,
            }
        )
    return in_maps


def kernel(x, ve, qkv_w, lambdas, proj_w):
    in_maps = _host_prep(x, ve, qkv_w, lambdas, proj_w)
    nc = _get_program()
    res = run_bass_kernel_spmd(nc, in_maps, list(range(N_CORES)))
    out = np.concatenate([res.results[c]["out"] for c in range(N_CORES)], axis=0)
    return out.reshape(B, T, DIM).astype(np.float32)


# ---------------------------------------------------------------------------
# Timing support (test.py only): run the program with device-resident inputs
# so repeated executions measure device time, and difference two repeat
# factors to cancel dispatch overhead.
# ---------------------------------------------------------------------------

def make_runner(in_maps, repeat=1):
    import jax
    import jax.numpy as jnp
    from jax.sharding import Mesh, PartitionSpec, NamedSharding
    from jax.experimental.shard_map import shard_map
    from concourse import bass2jax
    from concourse.bass2jax import _bass_exec_p, partition_id_tensor

    bass2jax.install_neuronx_cc_hook()
    nc = _build_program(repeat)

    in_names, out_names, out_avals, zero_outs = [], [], [], []
    partition_name = nc.partition_id_tensor.name if nc.partition_id_tensor else None
    for alloc in nc.m.functions[0].allocations:
        if not isinstance(alloc, mybir.MemoryLocationSet):
            continue
        name = alloc.memorylocations[0].name
        if alloc.kind == "ExternalInput":
            if name != partition_name:
                in_names.append(name)
        elif alloc.kind == "ExternalOutput":
            out_names.append(name)
            shape = tuple(alloc.tensor_shape)
            dtype = mybir.dt.np(alloc.dtype)
            out_avals.append(jax.core.ShapedArray(shape, dtype))
            zero_outs.append(np.zeros(shape, dtype))
    n_params = len(in_names)
    n_outs = len(out_avals)
    all_in_names = list(in_names) + out_names
    if partition_name is not None:
        all_in_names.append(partition_name)
    donate = tuple(range(n_params, n_params + n_outs))

    def _body(*args):
        operands = list(args)
        if partition_name is not None:
            operands.append(partition_id_tensor())
        outs = _bass_exec_p.bind(
            *operands,
            out_avals=tuple(out_avals),
            in_names=tuple(all_in_names),
            out_names=tuple(out_names),
            lowering_input_output_aliases=(),
            sim_require_finite=True,
            sim_require_nnan=True,
            nc=nc,
        )
        return tuple(outs)

    devices = jax.devices()[:N_CORES]
    mesh = Mesh(np.asarray(devices), ("core",))
    in_specs = (PartitionSpec("core"),) * (n_params + n_outs)
    out_specs = (PartitionSpec("core"),) * n_outs
    fn = jax.jit(
        shard_map(_body, mesh=mesh, in_specs=in_specs, out_specs=out_specs, check_rep=False),
        donate_argnums=donate,
        keep_unused=True,
    )
    sh = NamedSharding(mesh, PartitionSpec("core"))
    concat_in = [
        jax.device_put(
            np.concatenate([np.asarray(in_maps[c][nm]) for c in range(N_CORES)], axis=0), sh
        )
        for nm in in_names
    ]
    zero_shapes = [(N_CORES * z.shape[0], *z.shape[1:]) for z in zero_outs]
    zero_dtypes = [z.dtype for z in zero_outs]
    mkzeros = jax.jit(
        lambda: tuple(jnp.zeros(s, d) for s, d in zip(zero_shapes, zero_dtypes)),
        out_shardings=tuple(sh for _ in zero_shapes),
    )

    def run_once():
        zs = mkzeros()
        for z in zs:
            z.block_until_ready()
        outs = fn(*concat_in, *zs)
        for o in outs:
            o.block_until_ready()
        return outs

    return run_once


# revision 50
# speedup vs baseline: 1.0307x; 1.0307x over previous
"""Distributed Trainium2 Bass kernel for nn_CausalSelfAttention_66984309948568.

Strategy (8 NeuronCores, tensor-parallel over heads):
  - core h owns head h (8 heads, head_dim 128). All matmul operands bf16
    (full PE rate, half the DMA/SBUF traffic of f32); PSUM accumulation f32.
  - Phase 0 (per 512-wide t-chunk): qkv projection in [d, t] layout from a
    bf16 x^T chunk; v transposed to [t, d] tiles and fused with lambdas[1]*ve;
    q, k roped UNNORMALIZED (rope commutes with the per-t rms_norm scale).
    Row sums of q^2 / k^2 via ones-column matmuls, staged into [1, T] rows.
    Per-chunk Ln of the mean-square rows; a manually preloaded act-table
    set holding Ln+Exp+Copy makes every activation reload-free:
    rsq = exp(-0.5*ln(mean+eps) [+ ln 0.12 for q]).
  - Phase 2 (per chunk): normalize q/k chunk via ones-row broadcast matmul +
    elementwise mult; causal attention streamed per 512-column chunk in
    S^T = [s, t] layout (no max subtraction: |scores| <= 15.4); softmax
    denominator via accumulated ones-column matmul over exp(S^T); y
    normalized with broadcast reciprocal; bf16 result staged for AllToAll.
  - AllToAll (bf16) exchanges per-head y slices; each core projects its own
    512 output rows; host concatenates.
"""

import sys

sys.path.insert(0, "/opt/trn_rl_repo")

import numpy as np
import concourse.bass as bass
import concourse.bacc as bacc
import concourse.mybir as mybir
from concourse import tile
from concourse.bass_utils import run_bass_kernel_spmd

N_CORES = 8
B, T, DIM = 1, 4096, 1024
NUM_HEADS, HEAD_DIM = 8, 128
HDIM = NUM_HEADS * HEAD_DIM
SCALE = 0.12
EPS = 1.1920928955078125e-07
NCHUNK = T // 512          # 8 t-chunks of 512
NTT = T // 128             # 32 t-tiles of 128
TSLICE = T // N_CORES      # 512 output rows per core

f32 = mybir.dt.float32
bf16 = mybir.dt.bfloat16
FN = mybir.ActivationFunctionType
ALU = mybir.AluOpType
MASK_NEG = -30000.0


def _register_const(nc, value, dtype=f32):
    if (dtype, value) in nc.const_aps.aps:
        return
    t = nc.alloc_sbuf_tensor(f"const-{dtype.name}-{value}", [128, 1], dtype)
    nc.gpsimd.memset(t.ap(), value)
    nc.const_aps.aps[(dtype, value)] = t.ap()


def _act_combined_set_id(nc):
    # Index of the act-table set containing BOTH Ln and Exp (plus Copy):
    # preloading it lets the act-table pass skip every per-function reload
    # regardless of how the scheduler interleaves Ln/Exp.
    try:
        from concourse.hw_specs import get_activation_tables

        tabs = get_activation_tables(nc.m.arch)
        for idx, (name, funcs) in enumerate(tabs.items()):
            if FN.Ln in funcs and FN.Exp in funcs and FN.Copy in funcs:
                return idx
    except Exception:
        pass
    return 6  # natural_log_exp_and_others in the gen3 act_info.json


def _build_program(repeat=1):
    nc = bacc.Bacc(num_devices=N_CORES)
    _register_const(nc, EPS)
    _register_const(nc, float(np.log(SCALE)))
    _register_const(nc, 0.0)
    nc.all_engine_barrier()
    nc.scalar.add_instruction(
        mybir.InstLoadActFuncSet(
            name=nc.get_next_instruction_name(),
            act_func_set_id=_act_combined_set_id(nc),
            ins=[],
            outs=[],
        )
    )

    ln_scale_q = float(np.log(SCALE))

    # ---- DRAM parameters (per-core values supplied via in_maps) ----
    xt_d = nc.declare_dram_parameter("xt", [DIM, T], bf16, isOutput=False)
    wq_d = nc.declare_dram_parameter("wq", [128, DIM], bf16, isOutput=False)
    wk_d = nc.declare_dram_parameter("wk", [128, DIM], bf16, isOutput=False)
    wv_d = nc.declare_dram_parameter("wv", [128, DIM], bf16, isOutput=False)
    vew_d = nc.declare_dram_parameter("vew", [128, T], bf16, isOutput=False)
    cmat_d = nc.declare_dram_parameter("cmat", [128, T], bf16, isOutput=False)
    smat_d = nc.declare_dram_parameter("smat", [128, T], bf16, isOutput=False)
    mask_d = nc.declare_dram_parameter("maskc", [128, 128], f32, isOutput=False)
    pw_d = nc.declare_dram_parameter("pw", [128, 8 * DIM], bf16, isOutput=False)
    onc_d = nc.declare_dram_parameter("ones_col", [128, 1], bf16, isOutput=False)
    onr_d = nc.declare_dram_parameter("ones_row", [1, 128], bf16, isOutput=False)
    id_d = nc.declare_dram_parameter("ident", [128, 128], bf16, isOutput=False)
    out_d = nc.declare_dram_parameter("out", [TSLICE, DIM], f32, isOutput=True)

    with tile.TileContext(nc, num_cores=N_CORES) as tc:
        with (
            tc.tile_pool(name="persist", bufs=1) as persist,
            tc.tile_pool(name="dram", bufs=1, space="DRAM") as dram,
        ):
            # persistent SBUF tensors
            qT = persist.tile([128, T], bf16, tag="qT")       # rope(q), unnormalized
            kT = persist.tile([128, T], bf16, tag="kT")
            qnT = persist.tile([128, T], bf16, tag="qnT")     # 0.12 * norm-scaled
            knT = persist.tile([128, T], bf16, tag="knT")
            v_sb = persist.tile([128, NTT * 128], bf16, tag="v_sb")  # [t,d] tiles
            maskc = persist.tile([128, 128], f32, tag="maskc")
            onc = persist.tile([128, 1], bf16, tag="onc")
            onr = persist.tile([1, 128], bf16, tag="onr")
            ident = persist.tile([128, 128], bf16, tag="ident")
            lnr_q = persist.tile([1, T], f32, tag="lnr_q")    # ln(mean sq) rows
            lnr_k = persist.tile([1, T], f32, tag="lnr_k")
            rsq_q = persist.tile([1, T], bf16, tag="rsq_q")
            rsq_k = persist.tile([1, T], bf16, tag="rsq_k")
            pw = persist.tile([128, 8 * DIM], bf16, tag="pw")

            nc.gpsimd.dma_start(maskc[:], mask_d[:])
            nc.gpsimd.dma_start(onc[:], onc_d[:])
            nc.gpsimd.dma_start(onr[:], onr_d[:])
            nc.gpsimd.dma_start(ident[:], id_d[:])
            nc.sync.dma_start(pw[:], pw_d[:])

            a2a_in = dram.tile([N_CORES * 128, TSLICE], bf16, tag="a2a_in")
            a2a_out = dram.tile([N_CORES * 128, TSLICE], bf16, tag="a2a_out")

            # constant operands: loaded once, reused by every repeat
            wq = persist.tile([128, DIM], bf16, tag="wq")
            wk = persist.tile([128, DIM], bf16, tag="wk")
            wv = persist.tile([128, DIM], bf16, tag="wv")
            cmat = persist.tile([128, T], bf16, tag="cmat")
            smat = persist.tile([128, T], bf16, tag="smat")
            vew = persist.tile([128, T], bf16, tag="vew")
            ut = persist.tile([128, 512], bf16, tag="ut")
            nc.sync.dma_start(wq[:], wq_d[:])
            nc.sync.dma_start(wk[:], wk_d[:])
            nc.sync.dma_start(wv[:], wv_d[:])
            nc.gpsimd.dma_start(cmat[:], cmat_d[:])
            nc.gpsimd.dma_start(smat[:], smat_d[:])
            nc.gpsimd.dma_start(vew[:], vew_d[:])
            # rows 32:64 and 96:128 of ut stay zero forever: the rope
            # cross-term add runs over all 128 partitions in one op.
            nc.vector.memset(ut[:], 0.0)

            for _rep in range(repeat):
              # ============ Phase 0: qkv, rope (unnormalized), v ============
              with (
                  tc.tile_pool(name="xt", bufs=2) as xt_pool,
                  tc.tile_pool(name="qkv_ps", bufs=1, space=bass.MemorySpace.PSUM) as qkv_ps,
                  tc.tile_pool(name="row_ps", bufs=2, space=bass.MemorySpace.PSUM) as row_ps,
                  tc.tile_pool(name="tr_ps", bufs=2, space=bass.MemorySpace.PSUM) as tr_ps,
                  tc.tile_pool(name="evac", bufs=3) as evac,
                  tc.tile_pool(name="tmps", bufs=2) as tmps,
              ):

                  for c in range(NCHUNK):
                      cs = bass.ts(c, 512)
                      ps_q = qkv_ps.tile([128, 512], f32, tag="ps_q")
                      ps_k = qkv_ps.tile([128, 512], f32, tag="ps_k")
                      ps_v = qkv_ps.tile([128, 512], f32, tag="ps_v")
                      # one 1MB DMA per chunk: [p, dt, col] <- xt[128*dt + p, 512c + col]
                      xt_t = xt_pool.tile([128, 8, 512], bf16, tag="xt")
                      nc.sync.dma_start(
                          xt_t[:],
                          xt_d[:, 512 * c : 512 * (c + 1)].rearrange(
                              "(dt p) col -> p dt col", p=128
                          ),
                      )
                      for dt in range(8):
                          st, sp = dt == 0, dt == 7
                          nc.tensor.matmul(ps_q[:], wq[:, bass.ts(dt, 128)], xt_t[:, dt, :], start=st, stop=sp)
                          nc.tensor.matmul(ps_k[:], wk[:, bass.ts(dt, 128)], xt_t[:, dt, :], start=st, stop=sp)
                          nc.tensor.matmul(ps_v[:], wv[:, bass.ts(dt, 128)], xt_t[:, dt, :], start=st, stop=sp)

                      # ---- v: transpose [d,t]->[t,d] per 128-tile, add ve ----
                      vTc = evac.tile([128, 512], bf16, tag="vTc")
                      nc.scalar.copy(vTc[:], ps_v[:])
                      for j in range(4):
                          i = 4 * c + j
                          ps_t = tr_ps.tile([128, 128], bf16, tag="ps_t")
                          nc.tensor.transpose(ps_t[:], vTc[:, bass.ts(j, 128)], ident[:])
                          nc.vector.tensor_tensor(
                              v_sb[:, bass.ts(i, 128)], ps_t[:], vew[:, bass.ts(i, 128)], ALU.add
                          )

                      # ---- q, k: evac, square-rowsum, rope (unnormalized) ----
                      for which, ps_x, dstT in (("q", ps_q, qT), ("k", ps_k, kT)):
                          xTc = evac.tile([128, 512], bf16, tag="xTc")
                          nc.scalar.copy(xTc[:], ps_x[:])
                          sqc = tmps.tile([128, 512], bf16, tag="sqc")
                          nc.vector.tensor_tensor(sqc[:], xTc[:], xTc[:], ALU.mult)
                          ps_row = row_ps.tile([1, 512], f32, tag="ps_row")
                          nc.tensor.matmul(ps_row[:], onc[:], sqc[:], start=True, stop=True)
                          # ln(mean+eps) straight from PSUM; Ln lives in the
                          # same act table set as Copy, so phase 0 needs one
                          # table load total.
                          lnr_x = lnr_q if which == "q" else lnr_k
                          nc.scalar.activation(
                              lnr_x[:, cs], ps_row[:], FN.Ln,
                              bias=EPS, scale=1.0 / HEAD_DIM,
                          )
                          # rope: dstT = x*cmat + shift64(x)*smat
                          nc.vector.tensor_tensor(dstT[:, cs], xTc[:], cmat[:, cs], ALU.mult)
                          nc.vector.tensor_tensor(ut[0:32, :], xTc[64:96, :], smat[64:96, cs], ALU.mult)
                          nc.vector.tensor_tensor(ut[64:96, :], xTc[0:32, :], smat[0:32, cs], ALU.mult)
                          nc.vector.tensor_tensor(dstT[:, cs], dstT[:, cs], ut[:], ALU.add)

              # ================= Phase 2: causal attention ====================
              with (
                  tc.tile_pool(name="s_ps", bufs=4, space=bass.MemorySpace.PSUM) as s_ps,
                  tc.tile_pool(name="y_ps", bufs=2, space=bass.MemorySpace.PSUM) as y_ps,
                  tc.tile_pool(name="r_ps", bufs=1, space=bass.MemorySpace.PSUM) as r_ps,
                  tc.tile_pool(name="b_ps", bufs=1, space=bass.MemorySpace.PSUM) as b_ps,
                  tc.tile_pool(name="pt", bufs=4) as pt_pool,
                  tc.tile_pool(name="att_sb", bufs=2) as att_sb,
              ):
                  def normalize_chunk(cn):
                      """rsq = exp(-0.5*lnr [+ln 0.12]); qnT/knT = qT/kT * rsq."""
                      ns = bass.ts(cn, 512)
                      for lnr_x, bias, rsq, src, dst in (
                          (lnr_q, ln_scale_q, rsq_q, qT, qnT),
                          (lnr_k, 0.0, rsq_k, kT, knT),
                      ):
                          nc.scalar.activation(
                              rsq[:, ns], lnr_x[:, ns], FN.Exp,
                              bias=bias, scale=-0.5,
                          )
                          ps_b = b_ps.tile([128, 512], f32, tag="ps_b")
                          nc.tensor.matmul(ps_b[:], onr[:], rsq[:, ns], start=True, stop=True)
                          bb = att_sb.tile([128, 512], bf16, tag="bb")
                          nc.scalar.copy(bb[:], ps_b[:])
                          nc.vector.tensor_tensor(dst[:, ns], src[:, ns], bb[:], ALU.mult)

                  normalize_chunk(0)
                  normalize_chunk(1)
                  pending_finish = None
                  for c in range(NCHUNK):
                      cs = bass.ts(c, 512)
                      # normalize two chunks ahead: overlaps this chunk's
                      # attention and keeps the next chunks' qnT/knT ready
                      # well before their S matmuls.
                      if c + 2 < NCHUNK:
                          normalize_chunk(c + 2)

                      n_s = 4 * (c + 1)
                      ps_y = y_ps.tile([128, 512], f32, tag="ps_y")
                      ps_r = r_ps.tile([1, 512], f32, tag="ps_r")
                      # s-tile-granular software pipeline: S/exp of tile i
                      # run 2 tiles ahead of the y/r accumulation matmuls so
                      # the PE never stalls on the ACT exp latency.
                      DEPTH = 3
                      pts = {}

                      def emit_S(i):
                          # for a diagonal s-tile (j = i-4c >= 0) only the
                          # columns t_local >= 128j can be unmasked: stream the
                          # partial window and add the 128x128 triangle mask.
                          k_idx = i - 4 * c
                          o = 128 * k_idx if k_idx > 0 else 0
                          ps_S = s_ps.tile([128, 512], f32, tag="ps_S")
                          nc.tensor.matmul(
                              ps_S[:, o:512],
                              knT[:, bass.ts(i, 128)],
                              qnT[:, 512 * c + o : 512 * (c + 1)],
                              start=True, stop=True,
                          )
                          if k_idx >= 0:
                              nc.vector.tensor_tensor(
                                  ps_S[:, o : o + 128],
                                  ps_S[:, o : o + 128],
                                  maskc[:, 0:128],
                                  ALU.add,
                              )
                          pT = pt_pool.tile([128, 512], bf16, tag="pT")
                          nc.scalar.activation(pT[:, o:512], ps_S[:, o:512], FN.Exp)
                          pts[i] = (pT, o)

                      for i in range(min(DEPTH, n_s)):
                          emit_S(i)
                      # softmax denominator: non-diagonal exp tiles are summed
                      # in pairs on DVE so the ones-matmul streams half the
                      # columns; the r-matmul consumes each pair one pair LATE
                      # so the in-order PE queue never waits on the DVE add.
                      # Diagonal (partial-window) tiles keep per-tile matmuls.
                      pend = []      # deferred (sum_ap, region) for ps_r
                      even = None
                      r_started = False

                      def emit_r(ap, o, sp):
                          nonlocal r_started
                          nc.tensor.matmul(
                              ps_r[:, o:512], onc[:], ap,
                              start=not r_started, stop=sp,
                          )
                          r_started = True

                      for i in range(n_s):
                          if i + DEPTH < n_s:
                              emit_S(i + DEPTH)
                          if i == 2 and pending_finish is not None:
                              # previous chunk's normalize/ship chain runs here,
                              # a few tiles into this chunk, so its PE->DVE->PE
                              # round trip never stalls the chunk boundary.
                              pending_finish()
                              pending_finish = None
                          pT, o = pts.pop(i)
                          nc.tensor.matmul(
                              ps_y[:, o:512], v_sb[:, bass.ts(i, 128)], pT[:, o:512],
                              start=i == 0, stop=i == n_s - 1,
                          )
                          if i < 4 * c:          # non-diagonal: full window
                              if i % 2 == 0:
                                  even = pT
                              else:
                                  s2 = att_sb.tile([128, 512], bf16, tag="s2")
                                  nc.vector.tensor_tensor(
                                      s2[:], even[:], pT[:], ALU.add
                                  )
                                  while pend:
                                      emit_r(*pend.pop(0))
                                  pend.append((s2[:], 0, False))
                          else:                   # diagonal: per-tile, partial
                              while pend:
                                  emit_r(*pend.pop(0))
                              emit_r(pT[:, o:512], o, i == n_s - 1)
                      while pend:
                          emit_r(*pend.pop(0))

                      def finish_chunk(cf=c, ps_y_f=ps_y, ps_r_f=ps_r):
                          # normalize y chunk by 1/rowsum, ship to a2a buffer
                          rrec = att_sb.tile([1, 512], f32, tag="rrec")
                          nc.vector.reciprocal(rrec[:], ps_r_f[:])
                          rrecr = att_sb.tile([1, 512], bf16, tag="rrecr")
                          nc.vector.tensor_copy(rrecr[:], rrec[:])
                          ps_br = b_ps.tile([128, 512], f32, tag="ps_b")
                          nc.tensor.matmul(
                              ps_br[:], onr[:], rrecr[:], start=True, stop=True
                          )
                          brs = att_sb.tile([128, 512], f32, tag="brs")
                          nc.scalar.copy(brs[:], ps_br[:])
                          yn = att_sb.tile([128, 512], bf16, tag="yn")
                          nc.vector.tensor_tensor(yn[:], ps_y_f[:], brs[:], ALU.mult)
                          nc.scalar.dma_start(
                              a2a_in[128 * cf : 128 * (cf + 1), :], yn[:]
                          )

                      pending_finish = finish_chunk
                  pending_finish()

              nc.gpsimd.collective_compute(
                  "AllToAll",
                  ALU.bypass,
                  replica_groups=[list(range(N_CORES))],
                  ins=[a2a_in[:].opt()],
                  outs=[a2a_out[:].opt()],
              )

              # ================= Phase 3: output projection =====================
              with (
                  tc.tile_pool(name="proj_sb", bufs=1) as proj_sb,
                  tc.tile_pool(name="o_ps", bufs=2, space=bass.MemorySpace.PSUM) as o_ps,
                  tc.tile_pool(name="outp", bufs=3) as outp,
              ):
                  yT = proj_sb.tile([128, N_CORES * TSLICE], bf16, tag="yT")
                  nc.sync.dma_start(
                      yT[:].rearrange("p (h t) -> p h t", h=N_CORES),
                      a2a_out[:].rearrange("(h p) t -> p h t", p=128),
                  )
                  for m in range(4):
                      for dc in range(2):
                          ps_o = o_ps.tile([128, 512], f32, tag="ps_o")
                          for hh in range(8):
                              nc.tensor.matmul(
                                  ps_o[:],
                                  yT[:, hh * TSLICE + 128 * m : hh * TSLICE + 128 * (m + 1)],
                                  pw[:, hh * DIM + 512 * dc : hh * DIM + 512 * (dc + 1)],
                                  start=(hh == 0), stop=(hh == 7),
                              )
                          ob = outp.tile([128, 512], f32, tag="ob")
                          nc.vector.tensor_copy(ob[:], ps_o[:])
                          nc.scalar.dma_start(
                              out_d[128 * m : 128 * (m + 1), 512 * dc : 512 * (dc + 1)], ob[:]
                          )

    nc.finalize()
    return nc


_PROGRAM = None


def _get_program():
    global _PROGRAM
    if _PROGRAM is None:
        _PROGRAM = _build_program()
    return _PROGRAM


def _bf16(a):
    import ml_dtypes

    return np.asarray(a, dtype=np.float32).astype(ml_dtypes.bfloat16)


def _host_prep(x, ve, qkv_w, lambdas, proj_w):
    x = np.asarray(x, dtype=np.float32).reshape(T, DIM)
    ve = np.asarray(ve, dtype=np.float32).reshape(T, HDIM)
    qkv_w = np.asarray(qkv_w, dtype=np.float32)
    lam = np.asarray(lambdas, dtype=np.float32)
    proj_w = np.asarray(proj_w, dtype=np.float32)

    xt = _bf16(x.T)                                        # [DIM, T]

    # rope tables
    nfreq = HEAD_DIM // 4
    ang = (1.0 / 1024.0) ** np.linspace(0.0, 1.0, nfreq, dtype=np.float32)
    theta = np.arange(T, dtype=np.float32)[:, None] * ang[None, :]     # [T, 32]
    cosT = np.cos(theta).T.astype(np.float32)              # [32, T]
    sinT = np.sin(theta).T.astype(np.float32)
    cmat = np.empty((128, T), np.float32)
    smat = np.empty((128, T), np.float32)
    cmat[0:32] = cosT
    cmat[32:64] = 1.0
    cmat[64:96] = cosT
    cmat[96:128] = 1.0
    smat[0:32] = -sinT
    smat[32:64] = 0.0
    smat[64:96] = sinT
    smat[96:128] = 0.0

    # triangle mask for the diagonal 128x128 blocks (partial-window streams
    # never touch the fully-masked region to the left of the triangle)
    maskc = np.where(
        np.arange(128)[:, None] > np.arange(128)[None, :], MASK_NEG, 0.0
    ).astype(np.float32)

    ones_col = _bf16(np.ones((128, 1), np.float32))
    ones_row = _bf16(np.ones((1, 128), np.float32))
    ident = _bf16(np.eye(128, dtype=np.float32))

    in_maps = []
    for h in range(N_CORES):
        hs = slice(128 * h, 128 * (h + 1))
        # weight layout: w[p, dt*128 + m] = W[m, dt*128 + p]
        Wq = qkv_w[0, hs, :]                                # [128, DIM]
        Wk = qkv_w[1, hs, :]
        Wv = qkv_w[2, hs, :] * lam[0]
        def wlay(W):
            # [m, (dt p)] -> [p, (dt m)]
            a = W.reshape(128, 8, 128)                      # [m, dt, p]
            return _bf16(np.ascontiguousarray(a.transpose(2, 1, 0).reshape(128, DIM)))
        # vew[p, i*128 + c] = lam1 * ve[i*128 + p, h*128 + c]
        veh = (ve[:, hs] * lam[1]).reshape(NTT, 128, 128)   # [i, p, c]
        vew = np.ascontiguousarray(veh.transpose(1, 0, 2).reshape(128, T))
        # pw[p, n*DIM + D] = proj_w[D, 128n + p]
        pwh = proj_w.T.reshape(8, 128, DIM)                 # [n, e_p, D]
        pw = _bf16(np.ascontiguousarray(pwh.transpose(1, 0, 2).reshape(128, 8 * DIM)))
        in_maps.append(
            {
                "xt": xt,
                "wq": wlay(Wq),
                "wk": wlay(Wk),
                "wv": wlay(Wv),
                "vew": _bf16(vew),
                "cmat": _bf16(cmat),
                "smat": _bf16(smat),
                "maskc": maskc,
                "pw": pw,
                "ones_col": ones_col,
                "ones_row": ones_row,
                "ident": ident,
            }
        )
    return in_maps


def kernel(x, ve, qkv_w, lambdas, proj_w):
    in_maps = _host_prep(x, ve, qkv_w, lambdas, proj_w)
    nc = _get_program()
    res = run_bass_kernel_spmd(nc, in_maps, list(range(N_CORES)))
    out = np.concatenate([res.results[c]["out"] for c in range(N_CORES)], axis=0)
    return out.reshape(B, T, DIM).astype(np.float32)


# ---------------------------------------------------------------------------
# Timing support (test.py only): run the program with device-resident inputs
# so repeated executions measure device time, and difference two repeat
# factors to cancel dispatch overhead.
# ---------------------------------------------------------------------------

def make_runner(in_maps, repeat=1):
    import jax
    import jax.numpy as jnp
    from jax.sharding import Mesh, PartitionSpec, NamedSharding
    from jax.experimental.shard_map import shard_map
    from concourse import bass2jax
    from concourse.bass2jax import _bass_exec_p, partition_id_tensor

    bass2jax.install_neuronx_cc_hook()
    nc = _build_program(repeat)

    in_names, out_names, out_avals, zero_outs = [], [], [], []
    partition_name = nc.partition_id_tensor.name if nc.partition_id_tensor else None
    for alloc in nc.m.functions[0].allocations:
        if not isinstance(alloc, mybir.MemoryLocationSet):
            continue
        name = alloc.memorylocations[0].name
        if alloc.kind == "ExternalInput":
            if name != partition_name:
                in_names.append(name)
        elif alloc.kind == "ExternalOutput":
            out_names.append(name)
            shape = tuple(alloc.tensor_shape)
            dtype = mybir.dt.np(alloc.dtype)
            out_avals.append(jax.core.ShapedArray(shape, dtype))
            zero_outs.append(np.zeros(shape, dtype))
    n_params = len(in_names)
    n_outs = len(out_avals)
    all_in_names = list(in_names) + out_names
    if partition_name is not None:
        all_in_names.append(partition_name)
    donate = tuple(range(n_params, n_params + n_outs))

    def _body(*args):
        operands = list(args)
        if partition_name is not None:
            operands.append(partition_id_tensor())
        outs = _bass_exec_p.bind(
            *operands,
            out_avals=tuple(out_avals),
            in_names=tuple(all_in_names),
            out_names=tuple(out_names),
            lowering_input_output_aliases=(),
            sim_require_finite=True,
            sim_require_nnan=True,
            nc=nc,
        )
        return tuple(outs)

    devices = jax.devices()[:N_CORES]
    mesh = Mesh(np.asarray(devices), ("core",))
    in_specs = (PartitionSpec("core"),) * (n_params + n_outs)
    out_specs = (PartitionSpec("core"),) * n_outs
    fn = jax.jit(
        shard_map(_body, mesh=mesh, in_specs=in_specs, out_specs=out_specs, check_rep=False),
        donate_argnums=donate,
        keep_unused=True,
    )
    sh = NamedSharding(mesh, PartitionSpec("core"))
    concat_in = [
        jax.device_put(
            np.concatenate([np.asarray(in_maps[c][nm]) for c in range(N_CORES)], axis=0), sh
        )
        for nm in in_names
    ]
    zero_shapes = [(N_CORES * z.shape[0], *z.shape[1:]) for z in zero_outs]
    zero_dtypes = [z.dtype for z in zero_outs]
    mkzeros = jax.jit(
        lambda: tuple(jnp.zeros(s, d) for s, d in zip(zero_shapes, zero_dtypes)),
        out_shardings=tuple(sh for _ in zero_shapes),
    )

    def run_once():
        zs = mkzeros()
        for z in zs:
            z.block_until_ready()
        outs = fn(*concat_in, *zs)
        for o in outs:
            o.block_until_ready()
        return outs

    return run_once


# revision 59
# speedup vs baseline: 1.2876x; 1.2493x over previous
"""Distributed Trainium2 Bass kernel for nn_CausalSelfAttention_66984309948568.

Strategy (8 NeuronCores, tensor-parallel over heads):
  - core h owns head h (8 heads, head_dim 128). All matmul operands bf16
    (full PE rate, half the DMA/SBUF traffic of f32); PSUM accumulation f32.
  - Phase 0 (per 512-wide t-chunk): qkv projection in [d, t] layout from a
    bf16 x^T chunk; v transposed to [t, d] tiles and fused with lambdas[1]*ve;
    q, k roped UNNORMALIZED (rope commutes with the per-t rms_norm scale).
    Row sums of q^2 / k^2 via ones-column matmuls, staged into [1, T] rows.
    Per-chunk Ln of the mean-square rows; a manually preloaded act-table
    set holding Ln+Exp+Copy makes every activation reload-free:
    rsq = exp(-0.5*ln(mean+eps) [+ ln 0.12 for q]).
  - Phase 2 (per chunk): normalize q/k chunk via ones-row broadcast matmul +
    elementwise mult; causal attention streamed per 512-column chunk in
    S^T = [s, t] layout (no max subtraction: |scores| <= 15.4); softmax
    denominator via accumulated ones-column matmul over exp(S^T); y
    normalized with broadcast reciprocal; bf16 result staged for AllToAll.
  - AllToAll (bf16) exchanges per-head y slices; each core projects its own
    512 output rows; host concatenates.
"""

import sys

sys.path.insert(0, "/opt/trn_rl_repo")

import numpy as np
import concourse.bass as bass
import concourse.bacc as bacc
import concourse.mybir as mybir
from concourse import tile
from concourse.bass_utils import run_bass_kernel_spmd

N_CORES = 8
B, T, DIM = 1, 4096, 1024
NUM_HEADS, HEAD_DIM = 8, 128
HDIM = NUM_HEADS * HEAD_DIM
SCALE = 0.12
EPS = 1.1920928955078125e-07
NCHUNK = T // 512          # 8 t-chunks of 512
NTT = T // 128             # 32 t-tiles of 128
TSLICE = T // N_CORES      # 512 output rows per core

f32 = mybir.dt.float32
bf16 = mybir.dt.bfloat16
FN = mybir.ActivationFunctionType
ALU = mybir.AluOpType
MASK_NEG = -30000.0


def _register_const(nc, value, dtype=f32):
    if (dtype, value) in nc.const_aps.aps:
        return
    t = nc.alloc_sbuf_tensor(f"const-{dtype.name}-{value}", [128, 1], dtype)
    nc.gpsimd.memset(t.ap(), value)
    nc.const_aps.aps[(dtype, value)] = t.ap()


def _act_combined_set_id(nc):
    # Index of the act-table set containing BOTH Ln and Exp (plus Copy):
    # preloading it lets the act-table pass skip every per-function reload
    # regardless of how the scheduler interleaves Ln/Exp.
    try:
        from concourse.hw_specs import get_activation_tables

        tabs = get_activation_tables(nc.m.arch)
        for idx, (name, funcs) in enumerate(tabs.items()):
            if FN.Ln in funcs and FN.Exp in funcs and FN.Copy in funcs:
                return idx
    except Exception:
        pass
    return 6  # natural_log_exp_and_others in the gen3 act_info.json


def _build_program(repeat=1):
    nc = bacc.Bacc(num_devices=N_CORES)
    _register_const(nc, EPS)
    _register_const(nc, float(np.log(SCALE)))
    _register_const(nc, 0.0)
    nc.all_engine_barrier()
    nc.scalar.add_instruction(
        mybir.InstLoadActFuncSet(
            name=nc.get_next_instruction_name(),
            act_func_set_id=_act_combined_set_id(nc),
            ins=[],
            outs=[],
        )
    )

    ln_scale_q = float(np.log(SCALE))

    # ---- DRAM parameters (per-core values supplied via in_maps) ----
    xt_d = nc.declare_dram_parameter("xt", [DIM, T], bf16, isOutput=False)
    wq_d = nc.declare_dram_parameter("wq", [128, DIM], bf16, isOutput=False)
    wk_d = nc.declare_dram_parameter("wk", [128, DIM], bf16, isOutput=False)
    wv_d = nc.declare_dram_parameter("wv", [128, DIM], bf16, isOutput=False)
    vew_d = nc.declare_dram_parameter("vew", [128, T], bf16, isOutput=False)
    cmat_d = nc.declare_dram_parameter("cmat", [128, T], bf16, isOutput=False)
    smat_d = nc.declare_dram_parameter("smat", [128, T], bf16, isOutput=False)
    mask_d = nc.declare_dram_parameter("maskc", [128, 128], f32, isOutput=False)
    pw_d = nc.declare_dram_parameter("pw", [128, 8 * DIM], bf16, isOutput=False)
    onc_d = nc.declare_dram_parameter("ones_col", [128, 1], bf16, isOutput=False)
    onr_d = nc.declare_dram_parameter("ones_row", [1, 128], bf16, isOutput=False)
    id_d = nc.declare_dram_parameter("ident", [128, 128], bf16, isOutput=False)
    out_d = nc.declare_dram_parameter("out", [TSLICE, DIM], f32, isOutput=True)

    with tile.TileContext(nc, num_cores=N_CORES) as tc:
        with (
            tc.tile_pool(name="persist", bufs=1) as persist,
            tc.tile_pool(name="dram", bufs=1, space="DRAM") as dram,
        ):
            # persistent SBUF tensors
            qT = persist.tile([128, T], bf16, tag="qT")       # rope(q), unnormalized
            kT = persist.tile([128, T], bf16, tag="kT")
            qnT = persist.tile([128, T], bf16, tag="qnT")     # 0.12 * norm-scaled
            knT = persist.tile([128, T], bf16, tag="knT")
            v_sb = persist.tile([128, NTT * 128], bf16, tag="v_sb")  # [t,d] tiles
            maskc = persist.tile([128, 128], f32, tag="maskc")
            onc = persist.tile([128, 1], bf16, tag="onc")
            onr = persist.tile([1, 128], bf16, tag="onr")
            ident = persist.tile([128, 128], bf16, tag="ident")
            lnr_q = persist.tile([1, T], f32, tag="lnr_q")    # ln(mean sq) rows
            lnr_k = persist.tile([1, T], f32, tag="lnr_k")
            rsq_q = persist.tile([1, T], bf16, tag="rsq_q")
            rsq_k = persist.tile([1, T], bf16, tag="rsq_k")
            pw = persist.tile([128, 8 * DIM], bf16, tag="pw")

            nc.gpsimd.dma_start(maskc[:], mask_d[:])
            nc.gpsimd.dma_start(onc[:], onc_d[:])
            nc.gpsimd.dma_start(onr[:], onr_d[:])
            nc.gpsimd.dma_start(ident[:], id_d[:])
            nc.sync.dma_start(pw[:], pw_d[:])

            a2a_in = dram.tile([N_CORES * 128, TSLICE], bf16, tag="a2a_in")
            a2a_out = dram.tile([N_CORES * 128, TSLICE], bf16, tag="a2a_out")

            # constant operands: loaded once, reused by every repeat
            wq = persist.tile([128, DIM], bf16, tag="wq")
            wk = persist.tile([128, DIM], bf16, tag="wk")
            wv = persist.tile([128, DIM], bf16, tag="wv")
            cmat = persist.tile([128, T], bf16, tag="cmat")
            smat = persist.tile([128, T], bf16, tag="smat")
            vew = persist.tile([128, T], bf16, tag="vew")
            ut = persist.tile([128, 512], bf16, tag="ut")
            nc.sync.dma_start(wq[:], wq_d[:])
            nc.sync.dma_start(wk[:], wk_d[:])
            nc.sync.dma_start(wv[:], wv_d[:])
            nc.gpsimd.dma_start(cmat[:], cmat_d[:])
            nc.gpsimd.dma_start(smat[:], smat_d[:])
            nc.gpsimd.dma_start(vew[:], vew_d[:])
            # rows 32:64 and 96:128 of ut stay zero forever: the rope
            # cross-term add runs over all 128 partitions in one op.
            nc.vector.memset(ut[:], 0.0)

            for _rep in range(repeat):
              # ============ Phase 0: qkv, rope (unnormalized), v ============
              with (
                  tc.tile_pool(name="xt", bufs=2) as xt_pool,
                  tc.tile_pool(name="qkv_ps", bufs=1, space=bass.MemorySpace.PSUM) as qkv_ps,
                  tc.tile_pool(name="row_ps", bufs=2, space=bass.MemorySpace.PSUM) as row_ps,
                  tc.tile_pool(name="tr_ps", bufs=2, space=bass.MemorySpace.PSUM) as tr_ps,
                  tc.tile_pool(name="evac", bufs=3) as evac,
                  tc.tile_pool(name="tmps", bufs=2) as tmps,
              ):

                  for c in range(NCHUNK):
                      cs = bass.ts(c, 512)
                      ps_q = qkv_ps.tile([128, 512], f32, tag="ps_q")
                      ps_k = qkv_ps.tile([128, 512], f32, tag="ps_k")
                      ps_v = qkv_ps.tile([128, 512], f32, tag="ps_v")
                      # one 1MB DMA per chunk: [p, dt, col] <- xt[128*dt + p, 512c + col]
                      xt_t = xt_pool.tile([128, 8, 512], bf16, tag="xt")
                      nc.sync.dma_start(
                          xt_t[:],
                          xt_d[:, 512 * c : 512 * (c + 1)].rearrange(
                              "(dt p) col -> p dt col", p=128
                          ),
                      )
                      for dt in range(8):
                          st, sp = dt == 0, dt == 7
                          nc.tensor.matmul(ps_q[:], wq[:, bass.ts(dt, 128)], xt_t[:, dt, :], start=st, stop=sp)
                          nc.tensor.matmul(ps_k[:], wk[:, bass.ts(dt, 128)], xt_t[:, dt, :], start=st, stop=sp)
                          nc.tensor.matmul(ps_v[:], wv[:, bass.ts(dt, 128)], xt_t[:, dt, :], start=st, stop=sp)

                      # ---- v: transpose [d,t]->[t,d] per 128-tile, add ve ----
                      vTc = evac.tile([128, 512], bf16, tag="vTc")
                      nc.scalar.copy(vTc[:], ps_v[:])
                      for j in range(4):
                          i = 4 * c + j
                          ps_t = tr_ps.tile([128, 128], bf16, tag="ps_t")
                          nc.tensor.transpose(ps_t[:], vTc[:, bass.ts(j, 128)], ident[:])
                          nc.vector.tensor_tensor(
                              v_sb[:, bass.ts(i, 128)], ps_t[:], vew[:, bass.ts(i, 128)], ALU.add
                          )

                      # ---- q, k: evac, square-rowsum, rope (unnormalized) ----
                      for which, ps_x, dstT in (("q", ps_q, qT), ("k", ps_k, kT)):
                          xTc = evac.tile([128, 512], bf16, tag="xTc")
                          nc.scalar.copy(xTc[:], ps_x[:])
                          sqc = tmps.tile([128, 512], bf16, tag="sqc")
                          nc.vector.tensor_tensor(sqc[:], xTc[:], xTc[:], ALU.mult)
                          ps_row = row_ps.tile([1, 512], f32, tag="ps_row")
                          nc.tensor.matmul(ps_row[:], onc[:], sqc[:], start=True, stop=True)
                          # ln(mean+eps) straight from PSUM; Ln lives in the
                          # same act table set as Copy, so phase 0 needs one
                          # table load total.
                          lnr_x = lnr_q if which == "q" else lnr_k
                          nc.scalar.activation(
                              lnr_x[:, cs], ps_row[:], FN.Ln,
                              bias=EPS, scale=1.0 / HEAD_DIM,
                          )
                          # rope: dstT = x*cmat + shift64(x)*smat
                          nc.vector.tensor_tensor(dstT[:, cs], xTc[:], cmat[:, cs], ALU.mult)
                          nc.vector.tensor_tensor(ut[0:32, :], xTc[64:96, :], smat[64:96, cs], ALU.mult)
                          nc.vector.tensor_tensor(ut[64:96, :], xTc[0:32, :], smat[0:32, cs], ALU.mult)
                          nc.vector.tensor_tensor(dstT[:, cs], dstT[:, cs], ut[:], ALU.add)

              # ================= Phase 2: causal attention ====================
              with (
                  tc.tile_pool(name="s_ps", bufs=4, space=bass.MemorySpace.PSUM) as s_ps,
                  tc.tile_pool(name="y_ps", bufs=2, space=bass.MemorySpace.PSUM) as y_ps,
                  tc.tile_pool(name="r_ps", bufs=1, space=bass.MemorySpace.PSUM) as r_ps,
                  tc.tile_pool(name="b_ps", bufs=1, space=bass.MemorySpace.PSUM) as b_ps,
                  tc.tile_pool(name="pt", bufs=4) as pt_pool,
                  tc.tile_pool(name="att_sb", bufs=2) as att_sb,
              ):
                  def normalize_chunk(cn):
                      """rsq = exp(-0.5*lnr [+ln 0.12]); qnT/knT = qT/kT * rsq."""
                      ns = bass.ts(cn, 512)
                      for lnr_x, bias, rsq, src, dst in (
                          (lnr_q, ln_scale_q, rsq_q, qT, qnT),
                          (lnr_k, 0.0, rsq_k, kT, knT),
                      ):
                          nc.scalar.activation(
                              rsq[:, ns], lnr_x[:, ns], FN.Exp,
                              bias=bias, scale=-0.5,
                          )
                          ps_b = b_ps.tile([128, 512], f32, tag="ps_b")
                          nc.tensor.matmul(ps_b[:], onr[:], rsq[:, ns], start=True, stop=True)
                          bb = att_sb.tile([128, 512], bf16, tag="bb")
                          nc.vector.tensor_copy(bb[:], ps_b[:])
                          nc.vector.tensor_tensor(dst[:, ns], src[:, ns], bb[:], ALU.mult)

                  normalize_chunk(0)
                  normalize_chunk(1)
                  pending_finish = None
                  for c in range(NCHUNK):
                      cs = bass.ts(c, 512)
                      # normalize two chunks ahead: overlaps this chunk's
                      # attention and keeps the next chunks' qnT/knT ready
                      # well before their S matmuls.
                      if c + 2 < NCHUNK:
                          normalize_chunk(c + 2)

                      n_s = 4 * (c + 1)
                      ps_y = y_ps.tile([128, 512], f32, tag="ps_y")
                      ps_r = r_ps.tile([1, 512], f32, tag="ps_r")
                      # s-tile-granular S/exp pipeline running 3 tiles
                      # ahead of the y/r matmuls so the PE never stalls on
                      # the ACT exp latency. Diagonal s-tiles (j = i-4c >= 0)
                      # stream only the unmasked window t_local >= 128j.
                      DEPTH = 3
                      pts = {}

                      def emit_S(i):
                          k_idx = i - 4 * c
                          o = 128 * k_idx if k_idx > 0 else 0
                          ps_S = s_ps.tile([128, 512], f32, tag="ps_S")
                          nc.tensor.matmul(
                              ps_S[:, o:512],
                              knT[:, bass.ts(i, 128)],
                              qnT[:, 512 * c + o : 512 * (c + 1)],
                              start=True, stop=True,
                          )
                          if k_idx >= 0:
                              nc.vector.tensor_tensor(
                                  ps_S[:, o : o + 128],
                                  ps_S[:, o : o + 128],
                                  maskc[:, 0:128],
                                  ALU.add,
                              )
                          pT = pt_pool.tile([128, 512], bf16, tag="pT")
                          nc.scalar.activation(pT[:, o:512], ps_S[:, o:512], FN.Exp)
                          pts[i] = (pT[:], o)

                      for i in range(min(DEPTH, n_s)):
                          emit_S(i)
                      # softmax denominator: non-diagonal exp tiles are summed
                      # in pairs on DVE so the ones-matmul streams half the
                      # columns; the r-matmul consumes each pair one pair LATE
                      # so the in-order PE queue never waits on the DVE add.
                      # Diagonal (partial-window) tiles keep per-tile matmuls.
                      pend = []      # deferred (sum_ap, region) for ps_r
                      even = None
                      r_started = False

                      def emit_r(ap, o, sp):
                          nonlocal r_started
                          nc.tensor.matmul(
                              ps_r[:, o:512], onc[:], ap,
                              start=not r_started, stop=sp,
                          )
                          r_started = True

                      for i in range(n_s):
                          if i + DEPTH < n_s:
                              emit_S(i + DEPTH)
                          if i == 2 and pending_finish is not None:
                              # previous chunk's normalize/ship chain runs here,
                              # a few tiles into this chunk, so its PE->DVE->PE
                              # round trip never stalls the chunk boundary.
                              pending_finish()
                              pending_finish = None
                          pT, o = pts.pop(i)
                          nc.tensor.matmul(
                              ps_y[:, o:512], v_sb[:, bass.ts(i, 128)], pT[:, o:512],
                              start=i == 0, stop=i == n_s - 1,
                          )
                          if i < 4 * c:          # non-diagonal: full window
                              if i % 2 == 0:
                                  even = pT
                              else:
                                  s2 = att_sb.tile([128, 512], bf16, tag="s2")
                                  nc.vector.tensor_tensor(
                                      s2[:], even[:], pT[:], ALU.add
                                  )
                                  while pend:
                                      emit_r(*pend.pop(0))
                                  pend.append((s2[:], 0, False))
                          else:                   # diagonal: per-tile, partial
                              while pend:
                                  emit_r(*pend.pop(0))
                              emit_r(pT[:, o:512], o, i == n_s - 1)
                      while pend:
                          emit_r(*pend.pop(0))

                      def finish_chunk(cf=c, ps_y_f=ps_y, ps_r_f=ps_r):
                          # normalize y chunk by 1/rowsum, ship to a2a buffer
                          rrec = att_sb.tile([1, 512], f32, tag="rrec")
                          nc.vector.reciprocal(rrec[:], ps_r_f[:])
                          rrecr = att_sb.tile([1, 512], bf16, tag="rrecr")
                          nc.vector.tensor_copy(rrecr[:], rrec[:])
                          ps_br = b_ps.tile([128, 512], f32, tag="ps_b")
                          nc.tensor.matmul(
                              ps_br[:], onr[:], rrecr[:], start=True, stop=True
                          )
                          brs = att_sb.tile([128, 512], f32, tag="brs")
                          nc.vector.tensor_copy(brs[:], ps_br[:])
                          yn = att_sb.tile([128, 512], bf16, tag="yn")
                          nc.vector.tensor_tensor(yn[:], ps_y_f[:], brs[:], ALU.mult)
                          nc.scalar.dma_start(
                              a2a_in[128 * cf : 128 * (cf + 1), :], yn[:]
                          )

                      pending_finish = finish_chunk
                  pending_finish()

              nc.gpsimd.collective_compute(
                  "AllToAll",
                  ALU.bypass,
                  replica_groups=[list(range(N_CORES))],
                  ins=[a2a_in[:].opt()],
                  outs=[a2a_out[:].opt()],
              )

              # ================= Phase 3: output projection =====================
              with (
                  tc.tile_pool(name="proj_sb", bufs=1) as proj_sb,
                  tc.tile_pool(name="o_ps", bufs=2, space=bass.MemorySpace.PSUM) as o_ps,
                  tc.tile_pool(name="outp", bufs=3) as outp,
              ):
                  yT = proj_sb.tile([128, N_CORES * TSLICE], bf16, tag="yT")
                  nc.sync.dma_start(
                      yT[:].rearrange("p (h t) -> p h t", h=N_CORES),
                      a2a_out[:].rearrange("(h p) t -> p h t", p=128),
                  )
                  for m in range(4):
                      for dc in range(2):
                          ps_o = o_ps.tile([128, 512], f32, tag="ps_o")
                          for hh in range(8):
                              nc.tensor.matmul(
                                  ps_o[:],
                                  yT[:, hh * TSLICE + 128 * m : hh * TSLICE + 128 * (m + 1)],
                                  pw[:, hh * DIM + 512 * dc : hh * DIM + 512 * (dc + 1)],
                                  start=(hh == 0), stop=(hh == 7),
                              )
                          ob = outp.tile([128, 512], f32, tag="ob")
                          nc.vector.tensor_copy(ob[:], ps_o[:])
                          nc.scalar.dma_start(
                              out_d[128 * m : 128 * (m + 1), 512 * dc : 512 * (dc + 1)], ob[:]
                          )

    nc.finalize()
    return nc


_PROGRAM = None


def _get_program():
    global _PROGRAM
    if _PROGRAM is None:
        _PROGRAM = _build_program()
    return _PROGRAM


def _bf16(a):
    import ml_dtypes

    return np.asarray(a, dtype=np.float32).astype(ml_dtypes.bfloat16)


def _host_prep(x, ve, qkv_w, lambdas, proj_w):
    x = np.asarray(x, dtype=np.float32).reshape(T, DIM)
    ve = np.asarray(ve, dtype=np.float32).reshape(T, HDIM)
    qkv_w = np.asarray(qkv_w, dtype=np.float32)
    lam = np.asarray(lambdas, dtype=np.float32)
    proj_w = np.asarray(proj_w, dtype=np.float32)

    xt = _bf16(x.T)                                        # [DIM, T]

    # rope tables
    nfreq = HEAD_DIM // 4
    ang = (1.0 / 1024.0) ** np.linspace(0.0, 1.0, nfreq, dtype=np.float32)
    theta = np.arange(T, dtype=np.float32)[:, None] * ang[None, :]     # [T, 32]
    cosT = np.cos(theta).T.astype(np.float32)              # [32, T]
    sinT = np.sin(theta).T.astype(np.float32)
    cmat = np.empty((128, T), np.float32)
    smat = np.empty((128, T), np.float32)
    cmat[0:32] = cosT
    cmat[32:64] = 1.0
    cmat[64:96] = cosT
    cmat[96:128] = 1.0
    smat[0:32] = -sinT
    smat[32:64] = 0.0
    smat[64:96] = sinT
    smat[96:128] = 0.0

    # triangle mask for the diagonal 128x128 blocks (partial-window streams
    # never touch the fully-masked region to the left of the triangle)
    maskc = np.where(
        np.arange(128)[:, None] > np.arange(128)[None, :], MASK_NEG, 0.0
    ).astype(np.float32)

    ones_col = _bf16(np.ones((128, 1), np.float32))
    ones_row = _bf16(np.ones((1, 128), np.float32))
    ident = _bf16(np.eye(128, dtype=np.float32))

    in_maps = []
    for h in range(N_CORES):
        hs = slice(128 * h, 128 * (h + 1))
        # weight layout: w[p, dt*128 + m] = W[m, dt*128 + p]
        Wq = qkv_w[0, hs, :]                                # [128, DIM]
        Wk = qkv_w[1, hs, :]
        Wv = qkv_w[2, hs, :] * lam[0]
        def wlay(W):
            # [m, (dt p)] -> [p, (dt m)]
            a = W.reshape(128, 8, 128)                      # [m, dt, p]
            return _bf16(np.ascontiguousarray(a.transpose(2, 1, 0).reshape(128, DIM)))
        # vew[p, i*128 + c] = lam1 * ve[i*128 + p, h*128 + c]
        veh = (ve[:, hs] * lam[1]).reshape(NTT, 128, 128)   # [i, p, c]
        vew = np.ascontiguousarray(veh.transpose(1, 0, 2).reshape(128, T))
        # pw[p, n*DIM + D] = proj_w[D, 128n + p]
        pwh = proj_w.T.reshape(8, 128, DIM)                 # [n, e_p, D]
        pw = _bf16(np.ascontiguousarray(pwh.transpose(1, 0, 2).reshape(128, 8 * DIM)))
        in_maps.append(
            {
                "xt": xt,
                "wq": wlay(Wq),
                "wk": wlay(Wk),
                "wv": wlay(Wv),
                "vew": _bf16(vew),
                "cmat": _bf16(cmat),
                "smat": _bf16(smat),
                "maskc": maskc,
                "pw": pw,
                "ones_col": ones_col,
                "ones_row": ones_row,
                "ident": ident,
            }
        )
    return in_maps


def kernel(x, ve, qkv_w, lambdas, proj_w):
    in_maps = _host_prep(x, ve, qkv_w, lambdas, proj_w)
    nc = _get_program()
    res = run_bass_kernel_spmd(nc, in_maps, list(range(N_CORES)))
    out = np.concatenate([res.results[c]["out"] for c in range(N_CORES)], axis=0)
    return out.reshape(B, T, DIM).astype(np.float32)


# ---------------------------------------------------------------------------
# Timing support (test.py only): run the program with device-resident inputs
# so repeated executions measure device time, and difference two repeat
# factors to cancel dispatch overhead.
# ---------------------------------------------------------------------------

def make_runner(in_maps, repeat=1):
    import jax
    import jax.numpy as jnp
    from jax.sharding import Mesh, PartitionSpec, NamedSharding
    from jax.experimental.shard_map import shard_map
    from concourse import bass2jax
    from concourse.bass2jax import _bass_exec_p, partition_id_tensor

    bass2jax.install_neuronx_cc_hook()
    nc = _build_program(repeat)

    in_names, out_names, out_avals, zero_outs = [], [], [], []
    partition_name = nc.partition_id_tensor.name if nc.partition_id_tensor else None
    for alloc in nc.m.functions[0].allocations:
        if not isinstance(alloc, mybir.MemoryLocationSet):
            continue
        name = alloc.memorylocations[0].name
        if alloc.kind == "ExternalInput":
            if name != partition_name:
                in_names.append(name)
        elif alloc.kind == "ExternalOutput":
            out_names.append(name)
            shape = tuple(alloc.tensor_shape)
            dtype = mybir.dt.np(alloc.dtype)
            out_avals.append(jax.core.ShapedArray(shape, dtype))
            zero_outs.append(np.zeros(shape, dtype))
    n_params = len(in_names)
    n_outs = len(out_avals)
    all_in_names = list(in_names) + out_names
    if partition_name is not None:
        all_in_names.append(partition_name)
    donate = tuple(range(n_params, n_params + n_outs))

    def _body(*args):
        operands = list(args)
        if partition_name is not None:
            operands.append(partition_id_tensor())
        outs = _bass_exec_p.bind(
            *operands,
            out_avals=tuple(out_avals),
            in_names=tuple(all_in_names),
            out_names=tuple(out_names),
            lowering_input_output_aliases=(),
            sim_require_finite=True,
            sim_require_nnan=True,
            nc=nc,
        )
        return tuple(outs)

    devices = jax.devices()[:N_CORES]
    mesh = Mesh(np.asarray(devices), ("core",))
    in_specs = (PartitionSpec("core"),) * (n_params + n_outs)
    out_specs = (PartitionSpec("core"),) * n_outs
    fn = jax.jit(
        shard_map(_body, mesh=mesh, in_specs=in_specs, out_specs=out_specs, check_rep=False),
        donate_argnums=donate,
        keep_unused=True,
    )
    sh = NamedSharding(mesh, PartitionSpec("core"))
    concat_in = [
        jax.device_put(
            np.concatenate([np.asarray(in_maps[c][nm]) for c in range(N_CORES)], axis=0), sh
        )
        for nm in in_names
    ]
    zero_shapes = [(N_CORES * z.shape[0], *z.shape[1:]) for z in zero_outs]
    zero_dtypes = [z.dtype for z in zero_outs]
    mkzeros = jax.jit(
        lambda: tuple(jnp.zeros(s, d) for s, d in zip(zero_shapes, zero_dtypes)),
        out_shardings=tuple(sh for _ in zero_shapes),
    )

    def run_once():
        zs = mkzeros()
        for z in zs:
            z.block_until_ready()
        outs = fn(*concat_in, *zs)
        for o in outs:
            o.block_until_ready()
        return outs

    return run_once
